# revision 5
# baseline (speedup 1.0000x reference)
"""Trainium2 Bass kernel for AbstractLeakyRelu bound propagation.

Reference math (after collapsing the diagonal matmuls, valid because
uw, lw are always > 0 so sign-splitting diag(uw) etc. is exact):

    Wu = uw[:,None] * W,  Wl = lw[:,None] * W
    upper = uw * (Wp @ l0 + Wp' @ (u0-l0) ...)

Concretely with Wp = max(W, 0) and d = u0 - l0 >= 0:
    S_u = W @ l0 + Wp @ d        (== max(W,0)@u0 + min(W,0)@l0)
    S_l = W @ u0 - Wp @ d        (== max(W,0)@l0 + min(W,0)@u0)
    upper = uw * S_u + (ub_bias + uw * b)
    lower = lw * S_l + (lw * b)

Sharding: output-neuron dim (rows of W) split across 8 cores, 512 rows
each.  Each core streams its W^T shard ([4096, 512] fp32, pre-transposed
on host so the contraction dim lands on SBUF partitions), runs 2
accumulating matmuls per 128-row j-chunk (stationary = tiny vector
tiles), relu on the vector engine, and a 2-op epilogue.
"""

import os
import sys

import numpy as np

if "/opt/trn_rl_repo" not in sys.path:
    sys.path.insert(0, "/opt/trn_rl_repo")

N = 4096
M = 4096
NCORES = 8
ROWS = N // NCORES  # 512 output rows per core
NEG = np.float32(0.01)

# moving-operand dtype for the big matmuls: "float32r" is the PE's
# single-pass fp32 mode (1 cycle/row at free-dim >= 256 vs 4 for exact
# fp32).  Toggled off -> exact fp32 (4x slower on PE).
USE_F32R = True
CHUNK_ROWS = 1024  # j-rows per DMA chunk (must divide 4096, multiple of 128)

_CACHE = {}


def _build_nc(use_f32r, chunk_rows):
    import concourse.bacc as bacc
    import concourse.mybir as mybir
    import concourse.tile as tile

    f32 = mybir.dt.float32
    f32r = mybir.dt.float32r
    # fp32r must be typed end-to-end (walrus checkMatmultFP32r requires the
    # producer of every fp32r-matmul operand to output float32r)
    wd = f32r if use_f32r else f32

    nc = bacc.Bacc(
        trn_type="TRN2",
        target_bir_lowering=False,
        debug=False,
        num_devices=NCORES,
    )

    wt = nc.dram_tensor("wt", [M, ROWS], wd, kind="ExternalInput")
    lu = nc.dram_tensor("lu", [128, 2 * (M // 128)], wd, kind="ExternalInput")
    dd = nc.dram_tensor("dd", [128, 2 * (M // 128)], wd, kind="ExternalInput")
    uwlw = nc.dram_tensor("uwlw", [2, ROWS], f32, kind="ExternalInput")
    tb = nc.dram_tensor("tb", [2, ROWS], f32, kind="ExternalInput")
    out = nc.dram_tensor("out", [2, ROWS], f32, kind="ExternalOutput")

    n_chunks = M // chunk_rows
    sub = chunk_rows // 128  # 128-row j-chunks per DMA chunk
    cf = sub * ROWS  # free elements per W chunk tile
    n_q = M // 128  # total j-chunks (32)

    with tile.TileContext(nc) as tc:
        with (
            tc.tile_pool(name="w", bufs=3) as wpool,
            tc.tile_pool(name="wp", bufs=3) as wppool,
            tc.tile_pool(name="v", bufs=1) as vpool,
            tc.tile_pool(name="o", bufs=1) as opool,
            tc.tile_pool(name="ps", bufs=1, space="PSUM") as pspool,
        ):
            lut = vpool.tile([128, 2 * n_q], wd)
            nc.sync.dma_start(lut[:], lu.ap())
            ddt = vpool.tile([128, 2 * n_q], wd)
            nc.sync.dma_start(ddt[:], dd.ap())
            uwlwt = vpool.tile([2, ROWS], f32)
            nc.sync.dma_start(uwlwt[:], uwlw.ap())
            tbt = vpool.tile([2, ROWS], f32)
            nc.sync.dma_start(tbt[:], tb.ap())

            ps = pspool.tile([2, ROWS], f32)
            wt_ap = wt.ap()

            for cb in range(n_chunks):
                wtile = wpool.tile([128, cf], wd)
                src = wt_ap[cb * chunk_rows : (cb + 1) * chunk_rows, :].rearrange(
                    "(k p) i -> p k i", p=128
                )
                dst = wtile[:].rearrange("p (k i) -> p k i", k=sub)
                nc.sync.dma_start(dst, src)
                wptile = wppool.tile([128, cf], wd)
                nc.vector.tensor_scalar_max(wptile[:], wtile[:], 0.0)
                for k in range(sub):
                    q = cb * sub + k
                    nc.tensor.matmul(
                        ps[:],
                        lut[:, 2 * q : 2 * q + 2],
                        wtile[:, k * ROWS : (k + 1) * ROWS],
                        start=(q == 0),
                        stop=False,
                    )
                    nc.tensor.matmul(
                        ps[:],
                        ddt[:, 2 * q : 2 * q + 2],
                        wptile[:, k * ROWS : (k + 1) * ROWS],
                        start=False,
                        stop=(q == n_q - 1),
                    )

            tmp = opool.tile([2, ROWS], f32)
            nc.vector.tensor_mul(tmp[:], ps[:], uwlwt[:])
            outt = opool.tile([2, ROWS], f32)
            nc.vector.tensor_add(outt[:], tmp[:], tbt[:])
            nc.sync.dma_start(out.ap(), outt[:])

    nc.compile()
    return nc


def _get_nc():
    key = (USE_F32R, CHUNK_ROWS)
    if key not in _CACHE:
        _CACHE[key] = _build_nc(*key)
    return _CACHE[key]


def _host_prologue(lowerBound, upperBound, alpha, b):
    """Per-neuron slopes/biases, exactly mirroring the reference in fp32."""
    ns = NEG
    l = lowerBound.astype(np.float32)
    u = upperBound.astype(np.float32)
    lw = np.full_like(l, ns)
    uw = np.full_like(u, ns)
    positive = l > 0
    lw = np.where(positive, np.float32(1.0), lw)
    uw = np.where(positive, np.float32(1.0), uw)
    crossing = (l < 0) & (u > 0)
    denom = np.where(crossing, u - l, np.float32(1.0))
    slope = np.where(crossing, (u - ns * l) / denom, np.float32(0.0))
    uw = np.where(crossing, slope, uw)
    ub_bias = np.where(crossing, (ns - slope) * l, np.float32(0.0))
    a = np.clip(alpha.astype(np.float32), ns, np.float32(1.0))
    lw = np.where(crossing, a, lw)
    tb_u = (ub_bias + uw * b).astype(np.float32)
    tb_l = (lw * b).astype(np.float32)
    return lw.astype(np.float32), uw.astype(np.float32), tb_u, tb_l


def _interleave_pairs(x, y):
    """[M] , [M] -> [128, 2*(M//128)] with out[p, 2q+0]=x[q*128+p], out[p,2q+1]=y[...]."""
    xq = x.reshape(-1, 128).T  # [128, M//128]
    yq = y.reshape(-1, 128).T
    return np.stack([xq, yq], axis=2).reshape(128, -1).astype(np.float32)


def kernel(lowerBound, upperBound, alpha, W, b, l0, u0):
    from concourse.bass_utils import run_bass_kernel_spmd

    lowerBound = np.asarray(lowerBound, np.float32)
    upperBound = np.asarray(upperBound, np.float32)
    alpha = np.asarray(alpha, np.float32)
    W = np.asarray(W, np.float32)
    b = np.asarray(b, np.float32)
    l0 = np.asarray(l0, np.float32)
    u0 = np.asarray(u0, np.float32)

    lw, uw, tb_u, tb_l = _host_prologue(lowerBound, upperBound, alpha, b)
    d = (u0 - l0).astype(np.float32)

    # stationary tiles (shared across cores): psum row0 = lower, row1 = upper
    lu_np = _interleave_pairs(u0, l0)  # col0 -> W@u0 (lower), col1 -> W@l0 (upper)
    dd_np = _interleave_pairs(-d, d)  # col0 -> -Wp@d (lower), col1 -> +Wp@d (upper)

    wt_full = np.ascontiguousarray(W.T)  # [M, N]

    in_maps = []
    for k in range(NCORES):
        r = slice(k * ROWS, (k + 1) * ROWS)
        in_maps.append(
            {
                "wt": np.ascontiguousarray(wt_full[:, r]),
                "lu": lu_np,
                "dd": dd_np,
                "uwlw": np.ascontiguousarray(np.stack([lw[r], uw[r]])),
                "tb": np.ascontiguousarray(np.stack([tb_l[r], tb_u[r]])),
            }
        )

    nc = _get_nc()
    res = run_bass_kernel_spmd(nc, in_maps, core_ids=list(range(NCORES)))

    out = np.empty((2, N), np.float32)
    for k in range(NCORES):
        out[:, k * ROWS : (k + 1) * ROWS] = res.results[k]["out"]
    return out


# exposed for test.py profiling runs
def kernel_traced(lowerBound, upperBound, alpha, W, b, l0, u0, **trace_kwargs):
    from concourse.bass_utils import run_bass_kernel_spmd

    lw, uw, tb_u, tb_l = _host_prologue(
        np.asarray(lowerBound, np.float32),
        np.asarray(upperBound, np.float32),
        np.asarray(alpha, np.float32),
        np.asarray(b, np.float32),
    )
    W = np.asarray(W, np.float32)
    l0 = np.asarray(l0, np.float32)
    u0 = np.asarray(u0, np.float32)
    d = (u0 - l0).astype(np.float32)
    lu_np = _interleave_pairs(u0, l0)
    dd_np = _interleave_pairs(-d, d)
    wt_full = np.ascontiguousarray(W.T)
    in_maps = []
    for k in range(NCORES):
        r = slice(k * ROWS, (k + 1) * ROWS)
        in_maps.append(
            {
                "wt": np.ascontiguousarray(wt_full[:, r]),
                "lu": lu_np,
                "dd": dd_np,
                "uwlw": np.ascontiguousarray(np.stack([lw[r], uw[r]])),
                "tb": np.ascontiguousarray(np.stack([tb_l[r], tb_u[r]])),
            }
        )
    nc = _get_nc()
    res = run_bass_kernel_spmd(
        nc, in_maps, core_ids=list(range(NCORES)), trace=True, **trace_kwargs
    )
    out = np.empty((2, N), np.float32)
    for k in range(NCORES):
        out[:, k * ROWS : (k + 1) * ROWS] = res.results[k]["out"]
    return out, res


# revision 7
# speedup vs baseline: 1.2890x; 1.2890x over previous
"""Trainium2 Bass kernel for AbstractLeakyRelu bound propagation.

Math (diag matmuls collapsed; uw, lw in [0.01, 1] so sign-splits are exact):
    Wp = max(W, 0), d = u0 - l0 >= 0
    S_u = W @ l0 + Wp @ d       S_l = W @ u0 - Wp @ d
    upper = uw * S_u + (ub_bias + uw * b)
    lower = lw * S_l + (lw * b)

Sharding: output-neuron rows split across 8 cores (512 rows each).  The
host pre-transposes + prepacks each core's W^T shard into the exact SBUF
tile layout so every DMA descriptor is one long contiguous per-partition
run.  Each core: 8 x 1MB chunk DMAs (issued back-to-back on the Sync
HWDGE ring, FIFO-streamed), relu per chunk on the vector engine, and per
128-row j-chunk two fp32r accumulating matmuls with tiny stationary
vector tiles; PSUM [2, 512] ends up holding [S_l; S_u] directly.
"""

import os
import sys

import numpy as np

if "/opt/trn_rl_repo" not in sys.path:
    sys.path.insert(0, "/opt/trn_rl_repo")

N = 4096
M = 4096
NCORES = 8
ROWS = N // NCORES  # 512 output rows per core
NEG = np.float32(0.01)

USE_F32R = True
CHUNK_ROWS = 512  # j-rows per DMA chunk

_CACHE = {}


def _build_nc(use_f32r, chunk_rows):
    import concourse.bacc as bacc
    import concourse.mybir as mybir
    import concourse.tile as tile

    f32 = mybir.dt.float32
    f32r = mybir.dt.float32r
    # fp32r must be typed end-to-end (walrus checkMatmultFP32r requires the
    # producer of every fp32r-matmul operand to output float32r)
    wd = f32r if use_f32r else f32

    nc = bacc.Bacc(
        trn_type="TRN2",
        target_bir_lowering=False,
        debug=False,
        num_devices=NCORES,
    )

    n_chunks = M // chunk_rows
    sub = chunk_rows // 128  # 128-row j-chunks per DMA chunk
    cf = sub * ROWS  # free elements per W chunk tile
    n_q = M // 128  # total j-chunks (32)

    # W^T shard, host-prepacked to [n_chunks*128, cf] so chunk c's DMA is
    # a plain 2D [128, cf] with one contiguous run per partition.
    wt = nc.dram_tensor("wt", [n_chunks * 128, cf], wd, kind="ExternalInput")
    # stationary vectors: cols 0..2n_q-1 = [u0,l0] pairs, 2n_q.. = [-d,d] pairs
    lud = nc.dram_tensor("lud", [128, 4 * n_q], wd, kind="ExternalInput")
    # row0 = [lw | tb_l], row1 = [uw | tb_u]
    uwtb = nc.dram_tensor("uwtb", [2, 2 * ROWS], f32, kind="ExternalInput")
    out = nc.dram_tensor("out", [2, ROWS], f32, kind="ExternalOutput")

    with tile.TileContext(nc) as tc:
        with (
            tc.tile_pool(name="w", bufs=n_chunks) as wpool,
            tc.tile_pool(name="wp", bufs=3) as wppool,
            tc.tile_pool(name="v", bufs=1) as vpool,
            tc.tile_pool(name="o", bufs=1) as opool,
            tc.tile_pool(name="ps", bufs=1, space="PSUM") as pspool,
        ):
            # all W chunk DMAs first, in order, on the Sync HWDGE ring
            wtiles = []
            wt_ap = wt.ap()
            for cb in range(n_chunks):
                wtile = wpool.tile([128, cf], wd, name=f"wt{cb}", tag="wt")
                nc.sync.dma_start(wtile[:], wt_ap[cb * 128 : (cb + 1) * 128, :])
                wtiles.append(wtile)

            # small inputs on the Scalar HWDGE ring (parallel issuer)
            ludt = vpool.tile([128, 4 * n_q], wd)
            nc.scalar.dma_start(ludt[:], lud.ap())
            uwtbt = vpool.tile([2, 2 * ROWS], f32)
            nc.scalar.dma_start(uwtbt[:], uwtb.ap())

            ps = pspool.tile([2, ROWS], f32)

            for cb in range(n_chunks):
                wtile = wtiles[cb]
                wptile = wppool.tile([128, cf], wd)
                nc.vector.tensor_scalar_max(wptile[:], wtile[:], 0.0)
                for k in range(sub):
                    q = cb * sub + k
                    nc.tensor.matmul(
                        ps[:],
                        ludt[:, 2 * q : 2 * q + 2],
                        wtile[:, k * ROWS : (k + 1) * ROWS],
                        start=(q == 0),
                        stop=False,
                    )
                    nc.tensor.matmul(
                        ps[:],
                        ludt[:, 2 * n_q + 2 * q : 2 * n_q + 2 * q + 2],
                        wptile[:, k * ROWS : (k + 1) * ROWS],
                        start=False,
                        stop=(q == n_q - 1),
                    )

            tmp = opool.tile([2, ROWS], f32)
            nc.vector.tensor_mul(tmp[:], ps[:], uwtbt[:, :ROWS])
            outt = opool.tile([2, ROWS], f32)
            nc.vector.tensor_add(outt[:], tmp[:], uwtbt[:, ROWS:])
            nc.sync.dma_start(out.ap(), outt[:])

    nc.compile()
    return nc


def _get_nc():
    key = (USE_F32R, CHUNK_ROWS)
    if key not in _CACHE:
        _CACHE[key] = _build_nc(*key)
    return _CACHE[key]


def _host_prologue(lowerBound, upperBound, alpha, b):
    """Per-neuron slopes/biases, exactly mirroring the reference in fp32."""
    ns = NEG
    l = lowerBound.astype(np.float32)
    u = upperBound.astype(np.float32)
    lw = np.full_like(l, ns)
    uw = np.full_like(u, ns)
    positive = l > 0
    lw = np.where(positive, np.float32(1.0), lw)
    uw = np.where(positive, np.float32(1.0), uw)
    crossing = (l < 0) & (u > 0)
    denom = np.where(crossing, u - l, np.float32(1.0))
    slope = np.where(crossing, (u - ns * l) / denom, np.float32(0.0))
    uw = np.where(crossing, slope, uw)
    ub_bias = np.where(crossing, (ns - slope) * l, np.float32(0.0))
    a = np.clip(alpha.astype(np.float32), ns, np.float32(1.0))
    lw = np.where(crossing, a, lw)
    tb_u = (ub_bias + uw * b).astype(np.float32)
    tb_l = (lw * b).astype(np.float32)
    return lw.astype(np.float32), uw.astype(np.float32), tb_u, tb_l


def _interleave_pairs(x, y):
    """[M], [M] -> [128, 2*(M//128)]: out[p, 2q+c] = (x,y)[c][q*128+p]."""
    xq = x.reshape(-1, 128).T
    yq = y.reshape(-1, 128).T
    return np.stack([xq, yq], axis=2).reshape(128, -1).astype(np.float32)


def _prepare_inputs(lowerBound, upperBound, alpha, W, b, l0, u0):
    lowerBound = np.asarray(lowerBound, np.float32)
    upperBound = np.asarray(upperBound, np.float32)
    alpha = np.asarray(alpha, np.float32)
    W = np.ascontiguousarray(np.asarray(W, np.float32))
    b = np.asarray(b, np.float32)
    l0 = np.asarray(l0, np.float32)
    u0 = np.asarray(u0, np.float32)

    lw, uw, tb_u, tb_l = _host_prologue(lowerBound, upperBound, alpha, b)
    d = (u0 - l0).astype(np.float32)

    n_chunks = M // CHUNK_ROWS
    sub = CHUNK_ROWS // 128

    # psum row0 = lower (W@u0 - Wp@d), row1 = upper (W@l0 + Wp@d)
    lu_np = _interleave_pairs(u0, l0)
    dd_np = _interleave_pairs(-d, d)
    lud_np = np.ascontiguousarray(np.concatenate([lu_np, dd_np], axis=1))

    in_maps = []
    for k in range(NCORES):
        r = slice(k * ROWS, (k + 1) * ROWS)
        # pack W rows rk: W[r].T is [M, ROWS]; chunk c, partition p holds
        # rows c*CHUNK_ROWS + kk*128 + p for kk in range(sub), contiguous.
        wtk = W[r].T.reshape(n_chunks, sub, 128, ROWS)
        wtk = np.ascontiguousarray(wtk.transpose(0, 2, 1, 3)).reshape(
            n_chunks * 128, sub * ROWS
        )
        uwtb_np = np.empty((2, 2 * ROWS), np.float32)
        uwtb_np[0, :ROWS] = lw[r]
        uwtb_np[1, :ROWS] = uw[r]
        uwtb_np[0, ROWS:] = tb_l[r]
        uwtb_np[1, ROWS:] = tb_u[r]
        in_maps.append({"wt": wtk, "lud": lud_np, "uwtb": uwtb_np})
    return in_maps


def kernel(lowerBound, upperBound, alpha, W, b, l0, u0):
    from concourse.bass_utils import run_bass_kernel_spmd

    in_maps = _prepare_inputs(lowerBound, upperBound, alpha, W, b, l0, u0)
    nc = _get_nc()
    res = run_bass_kernel_spmd(nc, in_maps, core_ids=list(range(NCORES)))
    out = np.empty((2, N), np.float32)
    for k in range(NCORES):
        out[:, k * ROWS : (k + 1) * ROWS] = res.results[k]["out"]
    return out


# exposed for test.py profiling runs
def kernel_traced(lowerBound, upperBound, alpha, W, b, l0, u0, **trace_kwargs):
    from concourse.bass_utils import run_bass_kernel_spmd

    in_maps = _prepare_inputs(lowerBound, upperBound, alpha, W, b, l0, u0)
    nc = _get_nc()
    res = run_bass_kernel_spmd(
        nc, in_maps, core_ids=list(range(NCORES)), trace=True, **trace_kwargs
    )
    out = np.empty((2, N), np.float32)
    for k in range(NCORES):
        out[:, k * ROWS : (k + 1) * ROWS] = res.results[k]["out"]
    return out, res


# revision 23
# speedup vs baseline: 1.5359x; 1.1915x over previous
"""Trainium2 Bass kernel for AbstractLeakyRelu bound propagation.

Math (diag matmuls collapsed; uw, lw in [0.01, 1] so sign-splits are exact):
    Wp = max(W, 0), d = u0 - l0 >= 0
    S_u = W @ l0 + Wp @ d       S_l = W @ u0 - Wp @ d
    upper = uw * S_u + (ub_bias + uw * b)
    lower = lw * S_l + (lw * b)

Sharding: output-neuron rows split across 8 cores (512 rows each).  The
host pre-transposes + prepacks each core's W^T shard into the exact SBUF
tile layout so every DMA descriptor is one long contiguous per-partition
run.  Chunk DMAs are issued back-to-back on HWDGE ring(s) and stream
FIFO; relu runs per chunk on the vector engine; per 128-row j-chunk two
fp32r accumulating matmuls (stationary = tiny vector tiles) build PSUM
[2, 512] = [S_l; S_u] directly; 2-op epilogue.
"""

import os
import sys

import numpy as np

if "/opt/trn_rl_repo" not in sys.path:
    sys.path.insert(0, "/opt/trn_rl_repo")

# the device path runs through jax/PJRT on the axon platform; a cpu pin
# (sometimes used to keep the *reference* off the accelerator) would hide
# the NeuronCores from jax and break execution
if os.environ.get("JAX_PLATFORMS") == "cpu" and "jax" not in sys.modules:
    del os.environ["JAX_PLATFORMS"]

N = 4096
M = 4096
NCORES = 8
ROWS = N // NCORES  # 512 output rows per core
NEG = np.float32(0.01)

USE_F32R = True
# j-rows per DMA chunk (multiples of 128 summing to M); smaller tail
# chunks shrink the post-stream critical path
CHUNKS = (512, 512, 512, 512, 512, 512, 512, 256, 128, 128)
# W-chunk DMA issuer pattern, cycled: "s"=sync, "a"=scalar, "g"=gpsimd
RING = "s"
RELU_SPLIT = 1  # 0: one relu per chunk; 1: one relu per 128-row j-chunk
WARMUP = 0  # dummy PE matmuls at start to lift the HAM clock gate early

_CACHE = {}


def _cfg():
    return (USE_F32R, tuple(CHUNKS), RING, RELU_SPLIT, WARMUP)


def _build_nc(use_f32r, chunks, ring, relu_split, warmup):
    import concourse.bacc as bacc
    import concourse.mybir as mybir
    import concourse.tile as tile

    f32 = mybir.dt.float32
    f32r = mybir.dt.float32r
    wd = f32r if use_f32r else f32

    assert sum(chunks) == M and all(c % 128 == 0 for c in chunks)
    n_q = M // 128  # total 128-row j-chunks (32)

    nc = bacc.Bacc(
        trn_type="TRN2",
        target_bir_lowering=False,
        debug=False,
        num_devices=NCORES,
    )

    # flat prepacked W^T shard: chunk c = [128, (rows_c/128)*ROWS] contiguous
    wt = nc.dram_tensor("wt", [M * ROWS], wd, kind="ExternalInput")
    # stationary vectors: cols 0..2n_q-1 = [u0,l0] pairs, then [-d,d] pairs
    lud = nc.dram_tensor("lud", [128, 4 * n_q], wd, kind="ExternalInput")
    # cols 0:2 = I2, cols 2: = [tb_l/lw ; tb_u/uw]: a K=2 identity matmul
    # seeds PSUM with the (pre-divided) bias so the epilogue is one multiply
    tbr = nc.dram_tensor("tbr", [2, ROWS + 2], wd, kind="ExternalInput")
    # row0 = lw, row1 = uw
    uwtb = nc.dram_tensor("uwtb", [2, ROWS], f32, kind="ExternalInput")
    out = nc.dram_tensor("out", [2, ROWS], f32, kind="ExternalOutput")

    with tile.TileContext(nc) as tc:
        with (
            tc.tile_pool(name="w", bufs=len(chunks)) as wpool,
            tc.tile_pool(name="wp", bufs=4) as wppool,
            tc.tile_pool(name="v", bufs=1) as vpool,
            tc.tile_pool(name="o", bufs=1) as opool,
            tc.tile_pool(name="ps", bufs=1, space="PSUM") as pspool,
        ):
            if warmup:
                # PE sits idle ~12us (preamble + first chunk DMA); dummy
                # fp32 matmuls lift the HAM clock gate to 2.4 GHz before the
                # real fp32r stream arrives
                wsrc = vpool.tile([128, 128], f32)
                nc.gpsimd.memset(wsrc[:], 0.0)
                psw = pspool.tile([2, 128], f32, name="psw")
                for _ in range(warmup):
                    nc.tensor.matmul(
                        psw[:], wsrc[:, :2], wsrc[:], start=True, stop=True
                    )

            # small inputs on the Scalar HWDGE ring (tiny, complete early)
            ludt = vpool.tile([128, 4 * n_q], wd)
            nc.scalar.dma_start(ludt[:], lud.ap())
            tbrt = vpool.tile([2, ROWS + 2], wd)
            nc.scalar.dma_start(tbrt[:], tbr.ap())
            uwtbt = vpool.tile([2, ROWS], f32)
            nc.scalar.dma_start(uwtbt[:], uwtb.ap())

            # all W chunk DMAs, in order, on the HWDGE ring(s)
            wtiles = []
            wt_ap = wt.ap()
            off = 0
            engines = {"s": nc.sync, "a": nc.scalar, "g": nc.gpsimd}
            for cb, rows_c in enumerate(chunks):
                cf = (rows_c // 128) * ROWS
                wtile = wpool.tile([128, cf], wd, name=f"wt{cb}", tag="wt")
                src = wt_ap[off : off + 128 * cf].rearrange("(p f) -> p f", p=128)
                issuer = engines[ring[cb % len(ring)]]
                issuer.dma_start(wtile[:], src)
                wtiles.append(wtile)
                off += 128 * cf

            ps = pspool.tile([2, ROWS], f32)
            # seed psum with the pre-divided bias: ps = I2.T @ tbr = tbr
            nc.tensor.matmul(
                ps[:], tbrt[:, 0:2], tbrt[:, 2:], start=True, stop=False
            )

            q = 0
            for cb, rows_c in enumerate(chunks):
                sub = rows_c // 128
                wtile = wtiles[cb]
                cf = sub * ROWS
                wptile = wppool.tile([128, cf], wd)
                if relu_split:
                    for k in range(sub):
                        s = slice(k * ROWS, (k + 1) * ROWS)
                        nc.vector.tensor_scalar_max(wptile[:, s], wtile[:, s], 0.0)
                else:
                    nc.vector.tensor_scalar_max(wptile[:], wtile[:], 0.0)
                for k in range(sub):
                    nc.tensor.matmul(
                        ps[:],
                        ludt[:, 2 * q : 2 * q + 2],
                        wtile[:, k * ROWS : (k + 1) * ROWS],
                        start=False,
                        stop=False,
                    )
                    nc.tensor.matmul(
                        ps[:],
                        ludt[:, 2 * n_q + 2 * q : 2 * n_q + 2 * q + 2],
                        wptile[:, k * ROWS : (k + 1) * ROWS],
                        start=False,
                        stop=(q == n_q - 1),
                    )
                    q += 1

            outt = opool.tile([2, ROWS], f32)
            nc.vector.tensor_mul(outt[:], ps[:], uwtbt[:])
            nc.sync.dma_start(out.ap(), outt[:])

    nc.compile()
    return nc


def _get_nc():
    key = _cfg()
    if key not in _CACHE:
        _CACHE[key] = _build_nc(*key)
    return _CACHE[key]


def _host_prologue(lowerBound, upperBound, alpha, b):
    """Per-neuron slopes/biases, exactly mirroring the reference in fp32."""
    ns = NEG
    l = lowerBound.astype(np.float32)
    u = upperBound.astype(np.float32)
    lw = np.full_like(l, ns)
    uw = np.full_like(u, ns)
    positive = l > 0
    lw = np.where(positive, np.float32(1.0), lw)
    uw = np.where(positive, np.float32(1.0), uw)
    crossing = (l < 0) & (u > 0)
    denom = np.where(crossing, u - l, np.float32(1.0))
    slope = np.where(crossing, (u - ns * l) / denom, np.float32(0.0))
    uw = np.where(crossing, slope, uw)
    ub_bias = np.where(crossing, (ns - slope) * l, np.float32(0.0))
    a = np.clip(alpha.astype(np.float32), ns, np.float32(1.0))
    lw = np.where(crossing, a, lw)
    tb_u = (ub_bias + uw * b).astype(np.float32)
    tb_l = (lw * b).astype(np.float32)
    return lw.astype(np.float32), uw.astype(np.float32), tb_u, tb_l


def _interleave_pairs(x, y):
    """[M], [M] -> [128, 2*(M//128)]: out[p, 2q+c] = (x,y)[c][q*128+p]."""
    xq = x.reshape(-1, 128).T
    yq = y.reshape(-1, 128).T
    return np.stack([xq, yq], axis=2).reshape(128, -1).astype(np.float32)


def _prepare_inputs(lowerBound, upperBound, alpha, W, b, l0, u0):
    lowerBound = np.asarray(lowerBound, np.float32)
    upperBound = np.asarray(upperBound, np.float32)
    alpha = np.asarray(alpha, np.float32)
    W = np.ascontiguousarray(np.asarray(W, np.float32))
    b = np.asarray(b, np.float32)
    l0 = np.asarray(l0, np.float32)
    u0 = np.asarray(u0, np.float32)

    lw, uw, tb_u, tb_l = _host_prologue(lowerBound, upperBound, alpha, b)
    d = (u0 - l0).astype(np.float32)

    # psum row0 = lower (W@u0 - Wp@d), row1 = upper (W@l0 + Wp@d)
    lu_np = _interleave_pairs(u0, l0)
    dd_np = _interleave_pairs(-d, d)
    lud_np = np.ascontiguousarray(np.concatenate([lu_np, dd_np], axis=1))

    in_maps = []
    for k in range(NCORES):
        r = slice(k * ROWS, (k + 1) * ROWS)
        wtk_t = W[r].T  # [M, ROWS]
        # pack: for each chunk, [128, sub*ROWS] with partition p holding
        # j-rows {base + kk*128 + p : kk}, each row contiguous
        parts = []
        base = 0
        for rows_c in CHUNKS:
            sub = rows_c // 128
            blk = wtk_t[base : base + rows_c].reshape(sub, 128, ROWS)
            parts.append(np.ascontiguousarray(blk.transpose(1, 0, 2)).reshape(-1))
            base += rows_c
        wtk = np.concatenate(parts)
        uwtb_np = np.ascontiguousarray(np.stack([lw[r], uw[r]]))
        tbr_np = np.zeros((2, ROWS + 2), np.float32)
        tbr_np[0, 0] = 1.0
        tbr_np[1, 1] = 1.0
        tbr_np[0, 2:] = tb_l[r] / lw[r]
        tbr_np[1, 2:] = tb_u[r] / uw[r]
        in_maps.append(
            {"wt": wtk, "lud": lud_np, "uwtb": uwtb_np, "tbr": tbr_np}
        )
    return in_maps


def _run(in_maps, trace=False, **trace_kwargs):
    from concourse.bass_utils import run_bass_kernel_spmd

    nc = _get_nc()
    res = run_bass_kernel_spmd(
        nc, in_maps, core_ids=list(range(NCORES)), trace=trace, **trace_kwargs
    )
    out = np.empty((2, N), np.float32)
    for k in range(NCORES):
        out[:, k * ROWS : (k + 1) * ROWS] = res.results[k]["out"]
    return out, res


def kernel(lowerBound, upperBound, alpha, W, b, l0, u0):
    in_maps = _prepare_inputs(lowerBound, upperBound, alpha, W, b, l0, u0)
    return _run(in_maps, trace=False)[0]


def kernel_traced(lowerBound, upperBound, alpha, W, b, l0, u0, **trace_kwargs):
    in_maps = _prepare_inputs(lowerBound, upperBound, alpha, W, b, l0, u0)
    return _run(in_maps, trace=True, **trace_kwargs)
